# revision 9
# baseline (speedup 1.0000x reference)
"""Trainium2 Bass kernel for multi-head self-attention with Q=K=V=x@Wq.

Problem: x [4, 2048, 512] f32, Wq [512, 512] f32, HEAD=8 (head_dim=64).
  q = x @ Wq;  per (b, h): S = q_h q_h^T / 8; out = softmax(S) @ q_h.

Sharding (8 cores): core i -> batch b = i//2, head group g = i%2 (4 heads).
Each core gets x[b] [2048,512] and Wq[:, 256g:256g+256] [512,256]; produces
out[b, :, 256g:256g+256].  No cross-core communication.

On-core algorithm (per head pair, exploiting S symmetric since Q=K):
  - x_T via PE transposes; q_T[e, i] = (Wq^T x^T) via matmuls (e on partitions)
  - q_nat[j, d] per head via PE transposes of q_T
  - For each key block jb (128 rows a) and query half (1024 cols b):
      S[a, b] = q_T[:,a]^T q_T[:,b]  (2 matmuls N=512, K=64, head pair
      row-packed in the PE array via tile_position)
      E = exp(0.125 * S) via ScalarE ACT, PSUM->SBUF, with accum_out giving
      Z_a = sum_b E[a, b] for free (valid because S is symmetric: row sums
      equal column sums, and no max-subtraction is needed since diag(S)~8
      dominates and exp stays well within fp32 range)
      ctx_T[d, i] += q_nat[jb]^T E  accumulated over jb in PSUM (the head
      pair is column-packed: head0 -> partitions 0-63, head1 -> 64-127)
  - ctx_T -> SBUF, PE-transpose back to [i, d], multiply by 1/Z[i]
    (per-partition scalar), DMA out.
"""

import sys

sys.path.insert(0, "/opt/trn_rl_repo")

from contextlib import ExitStack

import numpy as np

import concourse.bass as bass
import concourse.tile as tile
from concourse import bacc, mybir
from concourse.masks import make_identity

B, S, D, HEAD = 4, 2048, 512, 8
HD = D // HEAD  # 64
EC = 256  # e-columns per core (4 heads)
F32 = mybir.dt.float32
N_CORES = 8

_PROGRAM = None


def build_program():
    nc = bacc.Bacc(None, target_bir_lowering=False)
    x_d = nc.dram_tensor("x", [S, D], F32, kind="ExternalInput")
    wq_d = nc.dram_tensor("wq", [D, EC], F32, kind="ExternalInput")
    out_d = nc.dram_tensor("out", [S, EC], F32, kind="ExternalOutput")

    x_r = x_d.rearrange("(ib p) d -> ib p d", p=128)  # [16, 128, 512]
    wq_r = wq_d.rearrange("(dc p) e -> p dc e", p=128)  # [128, 4, 256]
    out_r = out_d.rearrange("(ib p) e -> ib p e", p=128)  # [16, 128, 256]

    with tile.TileContext(nc) as tc, ExitStack() as ctx:
        sb = ctx.enter_context(tc.tile_pool(name="sb", bufs=1))
        xb = ctx.enter_context(tc.tile_pool(name="xb", bufs=4))
        ep = ctx.enter_context(tc.tile_pool(name="ep", bufs=4))
        ob = ctx.enter_context(tc.tile_pool(name="ob", bufs=4))
        ps = ctx.enter_context(tc.tile_pool(name="ps", bufs=2, space="PSUM"))
        cps = ctx.enter_context(tc.tile_pool(name="cps", bufs=1, space="PSUM"))

        ident = sb.tile([128, 128], F32)
        make_identity(nc, ident)

        wq_sb = sb.tile([128, 4, EC], F32)
        nc.sync.dma_start(out=wq_sb, in_=wq_r)

        x_T = sb.tile([128, 4, S], F32)  # [d_in_chunk, dc, i]
        q_T = sb.tile([128, 2, S], F32)  # [e_in_tile, et, i]
        q_nat = sb.tile([128, 4, 16, HD], F32)  # [j_in_block, h, jb, d]
        zacc = sb.tile([128, 2, 2, 2, 16], F32)  # [a, p, h2, half, jb]
        zsum = sb.tile([128, 2, 2, 16], F32)  # [a, p, h2, jb]
        rz = sb.tile([128, 2, 2, 16], F32)
        ctx_sb = sb.tile([128, 2, 2, 1024], F32)  # [d_pair, p, half, i]

        # ---- Phase 1: load x, build x_T via PE transposes ----
        for ib in range(16):
            xt = xb.tile([128, D], F32, tag="xt")
            nc.sync.dma_start(out=xt, in_=x_r[ib])
            for dc in range(4):
                tp = ps.tile([128, 128], F32, tag="ps")
                nc.tensor.transpose(tp, xt[:, dc * 128 : (dc + 1) * 128], ident)
                nc.vector.tensor_copy(x_T[:, dc, ib * 128 : (ib + 1) * 128], tp)

        # ---- Phase 2: q_T = Wq^T @ x^T  (e on partitions, i free) ----
        for et in range(2):
            for icc in range(4):
                qp = ps.tile([128, 512], F32, tag="ps")
                for dc in range(4):
                    nc.tensor.matmul(
                        qp,
                        wq_sb[:, dc, et * 128 : (et + 1) * 128],
                        x_T[:, dc, icc * 512 : (icc + 1) * 512],
                        start=(dc == 0),
                        stop=(dc == 3),
                    )
                nc.vector.tensor_copy(q_T[:, et, icc * 512 : (icc + 1) * 512], qp)

        # ---- Phase 3: q_nat per head via PE transposes of q_T ----
        for h in range(4):
            et, r = h // 2, (h % 2) * 64
            for jb in range(16):
                tp = ps.tile([128, HD], F32, tag="ps")
                nc.tensor.transpose(
                    tp,
                    q_T[r : r + 64, et, jb * 128 : (jb + 1) * 128],
                    ident[r : r + 64, r : r + 64],
                )
                nc.vector.tensor_copy(q_nat[:, h, jb, :], tp)

        # ---- Phase 4: attention main loop ----
        for p in range(2):  # head pair (heads 2p, 2p+1); q_T tile et = p
            for half in range(2):  # query-column half (i in [1024*half, +1024))
                # head h2 accumulates in its own bank pair: partitions
                # [64*h2, +64), columns [1024*h2, +1024) — two independent
                # PSUM accumulation groups must not share a bank, since
                # start=True clears has_written for the whole bank.
                cp = cps.tile([128, 2048], F32, tag="ctx")
                for jb in range(16):  # key block (rows a)
                    ebs = []
                    for h2 in range(2):
                        sp = ps.tile([128, 1024], F32, tag="ps")
                        lhs = q_T[h2 * 64 : (h2 + 1) * 64, p, jb * 128 : (jb + 1) * 128]
                        for nn in range(2):
                            rhs = q_T[
                                h2 * 64 : (h2 + 1) * 64,
                                p,
                                half * 1024 + nn * 512 : half * 1024 + (nn + 1) * 512,
                            ]
                            nc.tensor.matmul(
                                sp[:, nn * 512 : (nn + 1) * 512],
                                lhs,
                                rhs,
                                start=True,
                                stop=True,
                                tile_position=(h2 * 64, 0),
                            )
                        eb = ep.tile([128, 1024], F32, tag="eb")
                        nc.scalar.activation(
                            eb,
                            sp,
                            mybir.ActivationFunctionType.Exp,
                            scale=0.125,
                            accum_out=zacc[:, p, h2, half, jb : jb + 1],
                        )
                        ebs.append(eb)
                    for h2 in range(2):
                        for nn in range(2):
                            nc.tensor.matmul(
                                cp[
                                    h2 * 64 : (h2 + 1) * 64,
                                    h2 * 1024 + nn * 512 : h2 * 1024 + (nn + 1) * 512,
                                ],
                                q_nat[:, 2 * p + h2, jb, :],
                                ebs[h2][:, nn * 512 : (nn + 1) * 512],
                                start=(jb == 0),
                                stop=(jb == 15),
                                tile_position=(0, h2 * 64),
                            )
                for h2 in range(2):
                    nc.vector.tensor_copy(
                        ctx_sb[h2 * 64 : (h2 + 1) * 64, p, half, :],
                        cp[h2 * 64 : (h2 + 1) * 64, h2 * 1024 : (h2 + 1) * 1024],
                    )

        # ---- Phase 5: Z totals, reciprocal ----
        for p in range(2):
            for h2 in range(2):
                nc.vector.tensor_add(
                    zsum[:, p, h2, :], zacc[:, p, h2, 0, :], zacc[:, p, h2, 1, :]
                )
            nc.vector.reciprocal(rz[:, p], zsum[:, p])

        # ---- Phase 6: transpose ctx back to [i, d], normalize, store ----
        for p in range(2):
            for half in range(2):
                for icc in range(8):
                    ib = half * 8 + icc
                    tp = ps.tile([128, 128], F32, tag="ps")
                    nc.tensor.transpose(
                        tp, ctx_sb[:, p, half, icc * 128 : (icc + 1) * 128], ident
                    )
                    ot = ob.tile([128, 128], F32, tag="ot")
                    nc.vector.tensor_scalar_mul(
                        ot[:, 0:64], tp[:, 0:64], rz[:, p, 0, ib : ib + 1]
                    )
                    nc.vector.tensor_scalar_mul(
                        ot[:, 64:128], tp[:, 64:128], rz[:, p, 1, ib : ib + 1]
                    )
                    nc.sync.dma_start(
                        out=out_r[ib, :, p * 128 : (p + 1) * 128], in_=ot
                    )

    nc.compile()
    return nc


def get_program():
    global _PROGRAM
    if _PROGRAM is None:
        _PROGRAM = build_program()
    return _PROGRAM


def make_in_maps(x, Wq):
    x = np.asarray(x, dtype=np.float32)
    Wq = np.asarray(Wq, dtype=np.float32)
    in_maps = []
    for core in range(N_CORES):
        b, g = core // 2, core % 2
        in_maps.append(
            {
                "x": np.ascontiguousarray(x[b]),
                "wq": np.ascontiguousarray(Wq[:, g * EC : (g + 1) * EC]),
            }
        )
    return in_maps


def assemble(results):
    out = np.empty((B, S, D), dtype=np.float32)
    for core in range(N_CORES):
        b, g = core // 2, core % 2
        out[b, :, g * EC : (g + 1) * EC] = results[core]["out"]
    return out


def kernel(x, Wq):
    from concourse.bass_utils import run_bass_kernel_spmd

    nc = get_program()
    res = run_bass_kernel_spmd(nc, make_in_maps(x, Wq), list(range(N_CORES)))
    return assemble(res.results)
